# revision 12
# baseline (speedup 1.0000x reference)
"""DenseCRF mean-field (2,21,80,80) on 8 trn2 NeuronCores.

Math: msg = Q @ (3*Ks + 5*Kb) per batch, Q <- sigmoid(pred - msg), 5 iters.
 - Kb[n,m] = exp(-|f_n-f_m|^2/50) = d_n d_m exp(f_n.f_m/25), f in [0,1]^3.
   exp(f_n.f_m/25) is Taylor-expanded (order 2, rank-10 monomial feature
   map; truncation err ~3e-4 relative, far under the 2e-2 gate).
 - Ks = Ky kron Kx (separable Gaussian), applied exactly as two 80x80
   contractions.
 - Classes never mix => 42 (batch,class) rows split over 8 cores, no
   collectives. Each core: 6 class slots of one batch.

Per-core layouts (P = partition dim), all class-major, c in [0,6):
  state alternates  Y-layout [80(y), c*80+x]  /  X-layout [80(x), c*80+y]
  phiY [80(y), r*80+x], phiX [80(x), r*80+y]: monomial_r(f) (raw, bf16)
  phiM [10(r), y*80+x] via a DRAM bounce (partition-crossing relayout)
Iteration (y-type; x-type mirrors with x<->y), everything bf16 except the
f32r prediction add and the f32 psum accumulators:
  pf[x',c*80+y'] = i80.T @ predX            (1 matmul, f32r, opens psum)
  paT[x, c*80+y']= sum_y Q[y,cx] (-r3 Ky)[y,y']  (6 class matmuls: lhsT=Q)
  t[r,c]         = sum_n phi_r(n) Q[n,c]    (80 chunk matmuls, psum acc)
  ptS            = w_r * t                  (DVE tensor_scalar, bf16)
  axb            = copy(paT)                (psum->sbuf, split ACT/DVE)
  pf            += phiM[r,(y',x')] @ ptS    (80 matmuls, strided psum out)
  pf            += (+r3 Kx) @ axb           (1 matmul, closes psum)
  Qnext          = sigmoid(pf)              (1 ACT op, bf16; f32 last iter)
"""

import math

import numpy as np

B, C, H, W = 2, 21, 80, 80
N = H * W
ORDER = 2
GAMMA = 1.0 / 25.0
CW = 6    # class slots per core
FD = CW * 80  # 480, free dim of canonical state
NCORES = 8
NUM_ITERATIONS = 5
F32 = np.float32

HW_COLS = 240 + 240 + 80 + 80 + 1 + 480   # imY | imX | kyb | kxb | wco | predYb
HW_PHI = 641                              # first-DMA segment (through wco)
HF_COLS = 480 + 480 + 80                  # predY | predX | i80


def _feature_plan():
    """Monomial features of (f0,f1,f2) up to degree ORDER, canonical order.
    Returns (groups, weights): groups[i] = (parent_r, first_child_r,
    first_channel, n_children); weights[r] = -5 * gamma^k * multinom / k!."""
    idxs = [()]
    by_ix = {(): 0}
    cur = [()]
    for _k in range(1, ORDER + 1):
        new = []
        for ix in cur:
            start = ix[-1] if ix else 0
            for ch in range(start, 3):
                nix = ix + (ch,)
                by_ix[nix] = len(idxs)
                idxs.append(nix)
                new.append(nix)
        cur = new
    by_parent = {}
    for r, ix in enumerate(idxs):
        if r == 0:
            continue
        by_parent.setdefault(by_ix[ix[:-1]], []).append((r, ix[-1]))
    groups = []
    for pr, childs in sorted(by_parent.items()):
        r0, ch0 = childs[0]
        k = len(childs)
        assert [c for _, c in childs] == list(range(ch0, ch0 + k))
        assert [r for r, _ in childs] == list(range(r0, r0 + k))
        groups.append((pr, r0, ch0, k))
    weights = []
    for ix in idxs:
        k = len(ix)
        multinom = math.factorial(k)
        for ch in range(3):
            multinom //= math.factorial(ix.count(ch))
        weights.append(-5.0 * GAMMA**k * multinom / math.factorial(k))
    return groups, np.array(weights, dtype=F32)


_GROUPS, _WEIGHTS = _feature_plan()
R = len(_WEIGHTS)  # 10

_CLS_START = [0, 6, 12, 18]
_CLS_WIDTH = [6, 6, 6, 3]


def _spatial_1d(n):
    r = np.arange(n, dtype=np.float64)
    return np.exp(-((r[:, None] - r[None, :]) ** 2) / 18.0)


def _build_in_maps(predictions, image):
    import ml_dtypes
    bf = ml_dtypes.bfloat16
    predictions = np.asarray(predictions, dtype=F32)
    image = np.asarray(image, dtype=F32)
    ky = (-math.sqrt(3.0) * _spatial_1d(H)).astype(F32)
    kx = (+math.sqrt(3.0) * _spatial_1d(W)).astype(F32)
    i80 = np.eye(80, dtype=F32)
    in_maps = []
    for core in range(NCORES):
        b, g = divmod(core, 4)
        cls = (np.arange(CW) + _CLS_START[g]).clip(max=C - 1)
        psel = predictions[b, cls]                       # [CW, H, W] (c,y,x)
        predY = psel.transpose(1, 0, 2).reshape(H, FD)   # [y, c*80+x]
        predX = psel.transpose(2, 0, 1).reshape(W, FD)   # [x, c*80+y]
        imY = image[b].transpose(1, 0, 2).reshape(H, 240)  # [y, c*80+x]
        imX = image[b].transpose(2, 0, 1).reshape(W, 240)  # [x, c*80+y]

        bh = np.zeros((80, HW_COLS), dtype=bf)
        bh[:, 0:240] = imY.astype(bf)
        bh[:, 240:480] = imX.astype(bf)
        bh[:, 480:560] = ky.astype(bf)
        bh[:, 560:640] = kx.astype(bf)
        bh[0:R, 640] = _WEIGHTS.astype(bf)
        bh[:, 641:1121] = predY.astype(bf)
        bf32 = np.zeros((80, HF_COLS), dtype=F32)
        bf32[:, 0:480] = predY
        bf32[:, 480:960] = predX
        bf32[:, 960:1040] = i80
        in_maps.append({"bundh": bh, "bundf": bf32})
    return in_maps


def _assemble(results):
    out = np.zeros((B, C, H, W), dtype=F32)
    for core in range(NCORES):
        b, g = divmod(core, 4)
        w = _CLS_WIDTH[g]
        q = results[core]["qout"].reshape(W, CW, H)  # [x, c, y]
        out[b, _CLS_START[g]:_CLS_START[g] + w] = q[:, :w, :].transpose(1, 2, 0)
    return out


def _build_bass(n_iters=NUM_ITERATIONS):
    import concourse.bass as bass  # noqa: F401
    import concourse.mybir as mybir
    import concourse.tile as tile
    from concourse import bacc

    dt = mybir.dt
    AF = mybir.ActivationFunctionType

    nc = bacc.Bacc("TRN2", target_bir_lowering=False, debug=False)

    bundh_d = nc.dram_tensor("bundh", [80, HW_COLS], dt.bfloat16,
                             kind="ExternalInput")
    bundf_d = nc.dram_tensor("bundf", [80, HF_COLS], dt.float32r,
                             kind="ExternalInput")
    phid_d = nc.dram_tensor("phid", [R, N], dt.bfloat16, kind="Internal")
    qout_d = nc.dram_tensor("qout", [W, FD], dt.float32, kind="ExternalOutput")

    with tile.TileContext(nc) as tc:
        with (
            tc.tile_pool(name="const", bufs=1) as constp,
            tc.tile_pool(name="state", bufs=1) as statep,
            tc.tile_pool(name="work", bufs=2) as workp,
            tc.tile_pool(name="pf", bufs=2, space="PSUM") as pfp,
            tc.tile_pool(name="pa", bufs=2, space="PSUM") as pap,
            tc.tile_pool(name="pt", bufs=2, space="PSUM") as ptp,
        ):
            bh = constp.tile([80, HW_COLS], dt.bfloat16, tag="bundh")
            bfr = constp.tile([80, HF_COLS], dt.float32r, tag="bundf")
            # phi-feeding segment first so the build starts ASAP
            nc.sync.dma_start(bh[:, 0:HW_PHI], bundh_d[:][:, 0:HW_PHI])
            nc.sync.dma_start(bh[:, HW_PHI:HW_COLS],
                              bundh_d[:][:, HW_PHI:HW_COLS])
            nc.sync.dma_start(bfr[:], bundf_d[:])
            imY = bh[:, 0:240]
            imX = bh[:, 240:480]
            kyb = bh[:, 480:560]
            kxb = bh[:, 560:640]
            wcoH = bh[0:R, 640:641]
            predYb = bh[:, 641:1121]
            predYr = bfr[:, 0:480]
            predXr = bfr[:, 480:960]
            i80r = bfr[:, 960:1040]

            wco = constp.tile([R, 1], dt.float32, tag="wco")
            nc.vector.tensor_copy(wco[:], wcoH)
            # dummy sigmoid: forces the sigmoid act table (which also holds
            # copy) to be the one loaded, avoiding a mid-kernel table switch
            dummy = workp.tile([1, 1], dt.float32, tag="dummy")
            nc.scalar.activation(dummy[:], wco[0:1, 0:1], AF.Sigmoid)

            phiY = constp.tile([H, W * R], dt.bfloat16, tag="phiY")
            phiX = constp.tile([W, H * R], dt.bfloat16, tag="phiX")
            phiM = constp.tile([R, N], dt.bfloat16, tag="phiM")

            def build_phi(img, phi):
                # phi[p, r*80+u]: r0 = d = exp(-|f|^2/50) via a degree-2
                # Taylor of exp (|arg| <= 0.06 so err ~2e-6 abs); children
                # are parent*channel products, one DVE op per parent group.
                img3 = img.rearrange("p (c u) -> p c u", c=3)
                m = workp.tile([80, 240], dt.bfloat16, tag="m")
                s = workp.tile([80, 80], dt.bfloat16, tag="s")
                m3 = m[:].rearrange("p (c u) -> p c u", c=3)
                nc.vector.tensor_mul(m[:], img, img)
                nc.vector.tensor_add(s[:], m3[:, 0, :], m3[:, 1, :])
                nc.vector.tensor_add(s[:], s[:], m3[:, 2, :])
                phi3 = phi[:].rearrange("p (r u) -> p r u", r=R)
                d0 = phi3[:, 0, :]
                mult, add = mybir.AluOpType.mult, mybir.AluOpType.add
                # d = exp(u) ~ 1+u, u = -s/50 in [-0.06, 0]; 1.8e-3 rel err
                # on Kb, far under the gate, and two ops shorter on the
                # serial chain that gates the phiM bounce.
                nc.vector.tensor_scalar(d0, s[:], -1.0 / 50.0, 1.0, mult, add)
                for pr, r0, ch0, k in _GROUPS:
                    par = phi3[:, pr:pr + 1, :].broadcast_to([80, k, 80])
                    nc.vector.tensor_mul(phi3[:, r0:r0 + k, :], par,
                                         img3[:, ch0:ch0 + k, :])

            build_phi(imY, phiY)

            # phiM[r, y*80+x] via DRAM bounce: hop1 reorders (y,r,x)->(r,y,x)
            # into dram, hop2 is contiguous. Single hops: an extra split costs
            # more in serialized HWDGE setup than it saves in transfer overlap.
            phid_ryx = phid_d[:].rearrange("r (y x) -> y r x", x=80)
            phiY_ryx = phiY[:].rearrange("y (r x) -> y r x", r=R)
            nc.sync.dma_start(phid_ryx, phiY_ryx)
            nc.sync.dma_start(phiM[:], phid_d[:])

            qY = statep.tile([H, FD], dt.bfloat16, tag="qY")
            qX = statep.tile([W, FD], dt.bfloat16, tag="qX")
            qF = statep.tile([W, FD], dt.float32, tag="qF")
            axb = statep.tile([80, FD], dt.bfloat16, tag="axb")
            ptS = statep.tile([R, CW], dt.bfloat16, tag="ptS")

            def iteration(it):
                ytype = (it % 2 == 0)
                qin = predYb if it == 0 else (qY[:] if ytype else qX[:])
                qnext = qF if it == n_iters - 1 else (qX if ytype else qY)
                phiIn = phiY if ytype else phiX
                padd = predXr if ytype else predYr
                kA = kyb if ytype else kxb
                kB = kxb if ytype else kyb
                qcls = qin.rearrange("p (c u) -> p c u", c=CW)
                qch = qin.rearrange("p (c u) -> p u c", c=CW)
                phch = phiIn[:].rearrange("p (r u) -> p u r", r=R)

                pf = pfp.tile([80, FD], dt.float32, tag="pf")
                paT = pap.tile([80, FD], dt.float32, tag="paT")
                pt = ptp.tile([R, CW], dt.float32, tag="pt")

                # prediction add opens the psum accumulation group
                nc.tensor.matmul(pf[:], i80r, padd, start=True, stop=False,
                                 skip_group_check=True)
                # first spatial contraction, stationary = per-class state
                for c in range(CW):
                    nc.tensor.matmul(paT[:, c * 80:(c + 1) * 80],
                                     qcls[:, c, :], kA, start=True, stop=True)
                # bilateral gather
                for j in range(80):
                    nc.tensor.matmul(pt[:], phch[:, j, :], qch[:, j, :],
                                     start=(j == 0), stop=(j == 79))
                # relayout paT psum->sbuf, halves on ACT and DVE in parallel.
                # The explicit nosync edge keeps the scheduler from slotting
                # ptS (which waits on the slower mm1 accumulation) ahead of
                # the DVE half on the in-order DVE stream.
                nc.scalar.copy(axb[:, 0:240], paT[:, 0:240])
                cp = nc.vector.tensor_copy(axb[:, 240:FD], paT[:, 240:FD])
                pts_i = nc.vector.tensor_scalar_mul(ptS[:], pt[:], wco[:])
                import bass_rust as _br
                _br.add_dep_helper(pts_i.ins, cp.ins, sync=False,
                                   reason="keep ptS after axb copy on DVE")
                # bilateral scatter: strided psum out, c-interleaved.
                # Iteration 0 stalls on the phiM bounce, so there the second
                # spatial contraction (ready much earlier) goes first.
                pf3 = pf[:].rearrange("p (c u) -> p u c", c=CW)
                phiM3 = phiM[:].rearrange("r (y x) -> r x y", x=80)

                def scatter(last):
                    for j in range(80):
                        nc.tensor.matmul(pf3[:, j, :],
                                         (phiM[:, j * 80:(j + 1) * 80] if ytype
                                          else phiM3[:, j, :]), ptS[:],
                                         start=False, stop=(last and j == 79),
                                         skip_group_check=True)

                def spatial2(last):
                    nc.tensor.matmul(pf[:], kB, axb[:], start=False, stop=last,
                                     skip_group_check=True)

                if it == 0:
                    spatial2(False)
                    scatter(True)
                else:
                    scatter(False)
                    spatial2(True)
                nc.scalar.activation(qnext[:], pf[:], AF.Sigmoid)
                return qnext

            qfin = iteration(0)
            # phiX is first needed by iteration 1; the wait-ts keeps the
            # scheduler from hoisting it into the phiY-build/bounce window.
            with tc.tile_wait_until(0.005):
                build_phi(imX, phiX)
            for it in range(1, n_iters):
                qfin = iteration(it)

            nc.sync.dma_start(qout_d[:], qfin[:])

    nc.compile()
    return nc


def kernel(predictions, image):
    from concourse.bass_utils import run_bass_kernel_spmd

    nc = _build_bass()
    in_maps = _build_in_maps(predictions, image)
    last_err = None
    for _attempt in range(3):
        try:
            res = run_bass_kernel_spmd(nc, in_maps, core_ids=list(range(NCORES)))
            return _assemble(res.results)
        except Exception as e:  # transient device wedges happen; retry
            last_err = e
    raise last_err
